# revision 4
# baseline (speedup 1.0000x reference)
"""DCGRU cell (nn_DCGRUCell) Trainium2 Bass kernel.

Sharding: data-parallel over batch B=64 -> 8 batches per NeuronCore x 8 cores.
The two sparse diffusion supports (NNZ=65536 over 4096^2, ~64 nnz per 128x128
block => effectively half-dense at PE tile granularity) are densified on the
host into stationary-operand block layout and streamed from HBM; the spmm runs
as PSUM-accumulated dense block matmuls on the TensorEngine in bf16.

Per-core pipeline (all bf16 internal, fp32 final combine):
  gconv1: x1 = S_s @ x0 (pass A), then per dst tile: x2 = 2*S_s@x1 - x0,
  PE-transpose x_m tiles to feature-major, 5-term projection matmul with the
  row-reordered weight, fused bias+sigmoid on ScalarE. r multiplies the hx
  part of x0 in place (becoming gconv2's input); u spills to DRAM.
  gconv2 repeats with W_c/tanh; final u*hx + (1-u)*c in fp32, node-major.
"""

import os
import sys

import numpy as np

sys.path.insert(0, "/opt/trn_rl_repo")

import ml_dtypes

import concourse.bass as bass
import concourse.bacc as bacc
import concourse.tile as tile
from concourse import mybir
from concourse.bass_utils import run_bass_kernel_spmd
from concourse.masks import make_identity

BF16 = ml_dtypes.bfloat16

N = 4096
P = 128
NT = N // P          # 32 node tiles
F = 66               # features per batch (2 input + 64 state)
BL = 8               # batches per core
NS = 2               # supports
NC = 8               # cores
NU = 64              # num units
C512 = 64 * BL       # 512 = "state" columns per matmul
FB = F * BL          # 528

AF = mybir.ActivationFunctionType

LAST_RESULTS = None  # test harness introspection

_NC_CACHE = None


def _build_nc():
    nc = bacc.Bacc("TRN2", target_bir_lowering=False, debug=False)
    dt = mybir.dt

    x0_d = nc.dram_tensor("x0in", [N, F, BL], dt.bfloat16, kind="ExternalInput")
    hx_d = nc.dram_tensor("hxn", [N, BL, NU], dt.float32, kind="ExternalInput")
    s1_d = nc.dram_tensor("s1", [NS, NT, P, NT * P], dt.bfloat16, kind="ExternalInput")
    s2_d = nc.dram_tensor("s2", [NS, NT, P, NT * P], dt.bfloat16, kind="ExternalInput")
    wru_d = nc.dram_tensor("wru", [F, 5, 2 * NU], dt.bfloat16, kind="ExternalInput")
    wc_d = nc.dram_tensor("wc", [F, 5, NU], dt.bfloat16, kind="ExternalInput")
    bru_d = nc.dram_tensor("bru", [2 * NU, 1], dt.float32, kind="ExternalInput")
    bc_d = nc.dram_tensor("bc", [NU, 1], dt.float32, kind="ExternalInput")
    out_d = nc.dram_tensor("out", [BL, N * NU], dt.float32, kind="ExternalOutput")

    with tile.TileContext(nc) as tc:
        with (
            tc.tile_pool(name="const", bufs=1) as const,
            tc.tile_pool(name="xp", bufs=1) as xp,
            tc.tile_pool(name="dram", bufs=1, space="DRAM") as dram,
        ):
            ident = const.tile([P, P], dt.bfloat16, name="ident")
            make_identity(nc, ident)
            wru = const.tile([F, 5, 2 * NU], dt.bfloat16, name="wru_s")
            nc.sync.dma_start(out=wru, in_=wru_d.ap())
            wc = const.tile([F, 5, NU], dt.bfloat16, name="wc_s")
            nc.sync.dma_start(out=wc, in_=wc_d.ap())
            biasru = const.tile([2 * NU, 1], dt.float32, name="biasru")
            nc.sync.dma_start(out=biasru, in_=bru_d.ap())
            biasc = const.tile([NU, 1], dt.float32, name="biasc")
            nc.sync.dma_start(out=biasc, in_=bc_d.ap())

            usp = dram.tile([NT, P, BL * NU], dt.float32, name="uspill")

            x0_t = [xp.tile([P, F, BL], dt.bfloat16, name=f"x0_{t}") for t in range(NT)]
            x1_t = [
                [xp.tile([P, F, BL], dt.bfloat16, name=f"x1_{s}_{t}") for t in range(NT)]
                for s in range(NS)
            ]
            for t in range(NT):
                nc.sync.dma_start(out=x0_t[t], in_=x0_d.ap()[t * P : (t + 1) * P])

            def spmm_to_psum(ps, sch, xin_tiles):
                # ps[:, f, b] += sum_src S^T-block^T @ xin ; 512-col + 16-col split
                for k in range(NT):
                    nc.tensor.matmul(
                        ps[:, 0:64, :],
                        sch[:, k, :],
                        xin_tiles[k][:, 0:64, :],
                        start=(k == 0),
                        stop=(k == NT - 1),
                    )
                for k in range(NT):
                    nc.tensor.matmul(
                        ps[:, 64:F, :],
                        sch[:, k, :],
                        xin_tiles[k][:, 64:F, :],
                        start=(k == 0),
                        stop=(k == NT - 1),
                    )

            def pass_a(gi):
                # x1[s] = S_s @ x0 for both supports
                with (
                    tc.tile_pool(name=f"spA{gi}", bufs=2) as spool,
                    tc.tile_pool(name=f"psA{gi}", bufs=2, space="PSUM") as pspool,
                ):
                    for s in range(NS):
                        for t in range(NT):
                            sch = spool.tile([P, NT, P], dt.bfloat16, tag="schunk")
                            nc.sync.dma_start(out=sch, in_=s1_d.ap()[s, t])
                            ps = pspool.tile([P, F, BL], dt.float32, tag="ps_spmm")
                            spmm_to_psum(ps, sch, x0_t)
                            eng = nc.scalar.copy if (t % 2 == 0) else nc.vector.tensor_copy
                            eng(out=x1_t[s][t], in_=ps)

            def pass_b(gi):
                # per dst tile: x2 = 2*S@x1 - x0, transpose all 5 x_m to
                # feature-major, project, activate; gconv1 handles r/u,
                # gconv2 computes c and the final combine.
                is_ru = gi == 0
                with (
                    tc.tile_pool(name=f"spB{gi}", bufs=2) as spool,
                    tc.tile_pool(name=f"tp{gi}", bufs=2) as tpool,
                    tc.tile_pool(name=f"psB{gi}", bufs=1, space="PSUM") as psB,
                    tc.tile_pool(name=f"psT{gi}", bufs=2, space="PSUM") as psT,
                    tc.tile_pool(name=f"psV{gi}", bufs=2, space="PSUM") as psV,
                ):
                    for t in range(NT):
                        x2r = []
                        for s in range(NS):
                            sch2 = spool.tile([P, NT, P], dt.bfloat16, tag="schunk2")
                            nc.sync.dma_start(out=sch2, in_=s2_d.ap()[s, t])
                            ps2 = psB.tile([P, F, BL], dt.float32, tag="ps_x2")
                            spmm_to_psum(ps2, sch2, x1_t[s])
                            x2s = tpool.tile([P, F, BL], dt.bfloat16, tag=f"x2_{s}")
                            nc.vector.tensor_sub(x2s, ps2, x0_t[t])
                            x2r.append(x2s)

                        # order must match W row order: x0, x1_s0, x2_s0, x1_s1, x2_s1
                        msrc = [x0_t[t], x1_t[0][t], x2r[0], x1_t[1][t], x2r[1]]
                        xT = []
                        for m in range(5):
                            pT = psT.tile([F, BL, P], dt.bfloat16, tag="psT")
                            for b in range(BL):
                                nc.tensor.transpose(pT[:, b, :], msrc[m][:, :, b], ident)
                            xm = tpool.tile([F, BL, P], dt.bfloat16, tag="xT", bufs=6)
                            eng = nc.scalar.copy if (m % 2 == 0) else nc.vector.tensor_copy
                            eng(out=xm, in_=pT)
                            xT.append(xm)

                        odim = 2 * NU if is_ru else NU
                        wmat = wru if is_ru else wc
                        if is_ru:
                            for b in range(BL):
                                psv = psV.tile([2 * NU, P], dt.float32, tag="psv")
                                for m in range(5):
                                    nc.tensor.matmul(
                                        psv,
                                        wmat[:, m, :],
                                        xT[m][:, b, :],
                                        start=(m == 0),
                                        stop=(m == 4),
                                    )
                                sig = tpool.tile([2 * NU, P], dt.bfloat16, tag="sig")
                                nc.scalar.activation(sig, psv, AF.Sigmoid, bias=biasru)
                                # r -> x0 *= r (state part, in place: becomes gconv2 input)
                                pr = psT.tile([P, NU, 1], dt.bfloat16, tag="psT")
                                nc.tensor.transpose(
                                    pr[:, :, 0], sig[0:NU, :], ident[0:NU, 0:NU]
                                )
                                nc.vector.tensor_mul(
                                    x0_t[t][:, 2:F, b], x0_t[t][:, 2:F, b], pr[:, :, 0]
                                )
                                # u -> spill (fp32) for the final combine
                                pu = psT.tile([P, NU, 1], dt.bfloat16, tag="psT")
                                nc.tensor.transpose(
                                    pu[:, :, 0], sig[NU : 2 * NU, :], ident[NU:P, NU:P]
                                )
                                if b == 0:
                                    u_nm = tpool.tile([P, BL, NU], dt.float32, tag="u_nm")
                                nc.scalar.copy(out=u_nm[:, b, :], in_=pu[:, :, 0])
                            nc.sync.dma_start(out=usp[t], in_=u_nm)
                        else:
                            c_nm = tpool.tile([P, BL, NU], dt.float32, tag="c_nm")
                            for b in range(BL):
                                psv = psV.tile([NU, P], dt.float32, tag="psv")
                                for m in range(5):
                                    nc.tensor.matmul(
                                        psv,
                                        wmat[:, m, :],
                                        xT[m][:, b, :],
                                        start=(m == 0),
                                        stop=(m == 4),
                                    )
                                sigc = tpool.tile([NU, P], dt.bfloat16, tag="sig")
                                nc.scalar.activation(sigc, psv, AF.Tanh, bias=biasc)
                                pc = psT.tile([P, NU, 1], dt.bfloat16, tag="psT")
                                nc.tensor.transpose(
                                    pc[:, :, 0], sigc, ident[0:NU, 0:NU]
                                )
                                nc.scalar.copy(out=c_nm[:, b, :], in_=pc[:, :, 0])
                            # final: out = c + u * (hx - c)
                            hxt = tpool.tile([P, BL, NU], dt.float32, tag="hxt")
                            nc.sync.dma_start(
                                out=hxt, in_=hx_d.ap()[t * P : (t + 1) * P]
                            )
                            ut = tpool.tile([P, BL, NU], dt.float32, tag="ut")
                            nc.sync.dma_start(out=ut, in_=usp[t])
                            tmp = tpool.tile([P, BL, NU], dt.float32, tag="tmp")
                            nc.vector.tensor_sub(tmp, hxt, c_nm)
                            nc.vector.tensor_mul(tmp, tmp, ut)
                            osb = tpool.tile([P, BL, NU], dt.float32, tag="osb")
                            nc.vector.tensor_add(osb, tmp, c_nm)
                            dst = bass.AP(
                                out_d,
                                t * P * NU,
                                [[NU, P], [N * NU, BL], [1, NU]],
                            )
                            nc.sync.dma_start(out=dst, in_=osb)

            pass_a(0)
            pass_b(0)
            pass_a(1)
            pass_b(1)

    nc.finalize()
    return nc


def _pack_core(inputs_f32, hx_f32, b0):
    """Host-side per-core input packing (layout only, no model math)."""
    inp = inputs_f32[b0 : b0 + BL].reshape(BL, N, 2).transpose(1, 2, 0)  # [N,2,BL]
    hxs = hx_f32[b0 : b0 + BL].reshape(BL, N, NU)
    hx_f = hxs.transpose(1, 2, 0)  # [N,64,BL]
    x0 = np.concatenate([inp, hx_f], axis=1).astype(BF16)  # [N,66,BL]
    hx_nm = np.ascontiguousarray(hxs.transpose(1, 0, 2), dtype=np.float32)  # [N,BL,64]
    return x0, hx_nm


def make_in_maps(inputs, hx, sup_rows, sup_cols, sup_vals, W_ru, b_ru, W_c, b_c):
    inputs = np.asarray(inputs, dtype=np.float32)
    hx = np.asarray(hx, dtype=np.float32)
    sup_rows = np.asarray(sup_rows)
    sup_cols = np.asarray(sup_cols)
    sup_vals = np.asarray(sup_vals, dtype=np.float32)

    # densify supports (duplicate (r,c) pairs accumulate, matching segment_sum)
    s_dense = np.zeros((NS, N, N), dtype=np.float32)
    for s in range(NS):
        np.add.at(s_dense[s], (sup_rows[s], sup_cols[s]), sup_vals[s])

    def blockify(mat):
        # stationary layout: [dst, c_part, src, n] with
        # block[d, i, k, j] = mat[128d + j, 128k + i]  (= mat^T[c, n] blocks)
        t = mat.T.reshape(NT, P, NT, P)  # [k, i, d, j]
        return np.ascontiguousarray(
            t.transpose(2, 1, 0, 3).reshape(NT, P, NT * P)
        ).astype(BF16)

    s1 = np.stack([blockify(s_dense[s]) for s in range(NS)])
    s2 = np.stack([blockify(2.0 * s_dense[s]) for s in range(NS)])

    # W rows are interleaved (f, m) -> reorder to [f][m][o]
    wru = np.ascontiguousarray(
        np.asarray(W_ru, dtype=np.float32).reshape(F, 5, 2 * NU)
    ).astype(BF16)
    wc = np.ascontiguousarray(
        np.asarray(W_c, dtype=np.float32).reshape(F, 5, NU)
    ).astype(BF16)
    bru = np.asarray(b_ru, dtype=np.float32).reshape(2 * NU, 1)
    bc = np.asarray(b_c, dtype=np.float32).reshape(NU, 1)

    in_maps = []
    for c in range(NC):
        x0, hx_nm = _pack_core(inputs, hx, c * BL)
        in_maps.append(
            {
                "x0in": x0,
                "hxn": hx_nm,
                "s1": s1,
                "s2": s2,
                "wru": wru,
                "wc": wc,
                "bru": bru,
                "bc": bc,
            }
        )
    return in_maps


def kernel(inputs, hx, sup_rows, sup_cols, sup_vals, W_ru, b_ru, W_c, b_c):
    global LAST_RESULTS, _NC_CACHE

    in_maps = make_in_maps(
        inputs, hx, sup_rows, sup_cols, sup_vals, W_ru, b_ru, W_c, b_c
    )

    if _NC_CACHE is None:
        _NC_CACHE = _build_nc()
    nc = _NC_CACHE

    res = run_bass_kernel_spmd(nc, in_maps, core_ids=list(range(NC)), trace=False)
    LAST_RESULTS = res

    out = np.concatenate(
        [np.asarray(res.results[c]["out"], dtype=np.float32) for c in range(NC)],
        axis=0,
    )
    return out


# revision 17
# speedup vs baseline: 36.6636x; 36.6636x over previous
"""DCGRU cell (nn_DCGRUCell) Trainium2 Bass kernel.

Sharding: data-parallel over batch B=64 -> 8 batches per NeuronCore x 8 cores.
The two sparse diffusion supports (NNZ=65536 over 4096^2, ~64 nnz per 128x128
block => effectively half-dense at PE tile granularity) are densified on the
host into stationary-operand block layout and streamed from HBM; the spmm runs
as PSUM-accumulated dense block matmuls on the TensorEngine in bf16.

Per-core pipeline (all bf16 internal, fp32 final combine):
  gconv1: x1 = S_s @ x0 (pass A), then per dst tile: x2 = 2*S_s@x1 - x0,
  PE-transpose x_m tiles to feature-major, 5-term projection matmul with the
  row-reordered weight, fused bias+sigmoid on ScalarE. r multiplies the hx
  part of x0 in place (becoming gconv2's input); u spills to DRAM.
  gconv2 repeats with W_c/tanh; final u*hx + (1-u)*c in fp32, node-major.
"""

import os
import sys

import numpy as np

sys.path.insert(0, "/opt/trn_rl_repo")

import ml_dtypes

import concourse.bass as bass
import concourse.bacc as bacc
import concourse.tile as tile
from concourse import mybir
from concourse.bass_utils import run_bass_kernel_spmd
from concourse.masks import make_identity

BF16 = ml_dtypes.bfloat16

N = 4096
P = 128
NT = N // P          # 32 node tiles
F = 66               # features per batch (2 input + 64 state)
BL = 8               # batches per core
NS = 2               # supports
NC = 8               # cores
NU = 64              # num units
C512 = 64 * BL       # 512 = "state" columns per matmul
FB = F * BL          # 528

AF = mybir.ActivationFunctionType

LAST_RESULTS = None  # test harness introspection

_NC_CACHE = None


def _build_nc():
    nc = bacc.Bacc("TRN2", target_bir_lowering=False, debug=False)
    dt = mybir.dt

    x0_d = nc.dram_tensor("x0in", [N, F, BL], dt.bfloat16, kind="ExternalInput")
    hx_d = nc.dram_tensor("hxn", [N, BL, NU], dt.float32, kind="ExternalInput")
    s1_d = nc.dram_tensor("s1", [NS, NT, P, NT * P], dt.bfloat16, kind="ExternalInput")
    s2_d = nc.dram_tensor("s2", [NS, NT, P, NT * P], dt.bfloat16, kind="ExternalInput")
    wru_d = nc.dram_tensor("wru", [F, 5, 2 * NU], dt.bfloat16, kind="ExternalInput")
    wc_d = nc.dram_tensor("wc", [F, 5, NU], dt.bfloat16, kind="ExternalInput")
    bru_d = nc.dram_tensor("bru", [2 * NU, 1], dt.float32, kind="ExternalInput")
    bc_d = nc.dram_tensor("bc", [NU, 1], dt.float32, kind="ExternalInput")
    out_d = nc.dram_tensor("out", [BL, N * NU], dt.float32, kind="ExternalOutput")

    with tile.TileContext(nc) as tc:
        with (
            tc.tile_pool(name="const", bufs=1) as const,
            tc.tile_pool(name="xp", bufs=1) as xp,
            tc.tile_pool(name="dram", bufs=1, space="DRAM") as dram,
        ):
            ident = const.tile([P, P], dt.bfloat16, name="ident")
            make_identity(nc, ident)
            wru = const.tile([F, 5, 2 * NU], dt.bfloat16, name="wru_s")
            nc.sync.dma_start(out=wru, in_=wru_d.ap())
            wc = const.tile([F, 5, NU], dt.bfloat16, name="wc_s")
            nc.sync.dma_start(out=wc, in_=wc_d.ap())
            biasru = const.tile([2 * NU, 1], dt.float32, name="biasru")
            nc.sync.dma_start(out=biasru, in_=bru_d.ap())
            biasc = const.tile([NU, 1], dt.float32, name="biasc")
            nc.sync.dma_start(out=biasc, in_=bc_d.ap())

            usp = dram.tile([NT, P, BL * NU], dt.float32, name="uspill")

            x0_t = [xp.tile([P, F, BL], dt.bfloat16, name=f"x0_{t}") for t in range(NT)]
            x1_t = [
                [xp.tile([P, F, BL], dt.bfloat16, name=f"x1_{s}_{t}") for t in range(NT)]
                for s in range(NS)
            ]
            # gconv1's x2 input-part columns, reused by gconv2
            x2i_t = [
                [xp.tile([P, 2, BL], dt.bfloat16, name=f"x2i_{s}_{t}") for t in range(NT)]
                for s in range(NS)
            ]
            for t in range(NT):
                nc.sync.dma_start(out=x0_t[t], in_=x0_d.ap()[t * P : (t + 1) * P])

            def spmm_to_psum(ps, sch, xin_tiles, inp_part=True):
                # ps[:, f, b] += sum_src S^T-block^T @ xin ; 512-col + 16-col split
                for k in range(NT):
                    nc.tensor.matmul(
                        ps[:, 0:64, :],
                        sch[:, k, :],
                        xin_tiles[k][:, 0:64, :],
                        start=(k == 0),
                        stop=(k == NT - 1),
                    )
                if not inp_part:
                    # gconv2: the 2 input features are unchanged from gconv1,
                    # so their diffusion is reused instead of recomputed
                    return
                for k in range(NT):
                    nc.tensor.matmul(
                        ps[:, 64:F, :],
                        sch[:, k, :],
                        xin_tiles[k][:, 64:F, :],
                        start=(k == 0),
                        stop=(k == NT - 1),
                    )

            def pass_a(gi):
                # x1[s] = S_s @ x0 for both supports. In gconv2 only the state
                # part is recomputed; x1's input-part columns keep gconv1's
                # values (identical by construction).
                first = gi == 0
                with (
                    tc.tile_pool(name=f"spA{gi}", bufs=2) as spool,
                    tc.tile_pool(name=f"psA{gi}", bufs=2, space="PSUM") as pspool,
                ):
                    for s in range(NS):
                        for t in range(NT):
                            sch = spool.tile([P, NT, P], dt.bfloat16, tag="schunk")
                            nc.sync.dma_start(out=sch, in_=s1_d.ap()[s, t])
                            ps = pspool.tile([P, F, BL], dt.float32, tag="ps_spmm")
                            spmm_to_psum(ps, sch, x0_t, inp_part=first)
                            eng = nc.scalar.copy if (t % 2 == 0) else nc.vector.tensor_copy
                            if first:
                                eng(out=x1_t[s][t], in_=ps)
                            else:
                                eng(out=x1_t[s][t][:, 0:64, :], in_=ps[:, 0:64, :])

            def pass_b(gi):
                # per dst tile: x2 = 2*S@x1 - x0, transpose all 5 x_m to
                # feature-major, project, activate; gconv1 handles r/u,
                # gconv2 computes c and the final combine.
                is_ru = gi == 0
                with (
                    tc.tile_pool(name=f"spB{gi}", bufs=2) as spool,
                    tc.tile_pool(name=f"tp{gi}", bufs=2) as tpool,
                    tc.tile_pool(name=f"psB{gi}", bufs=1, space="PSUM") as psB,
                    tc.tile_pool(name=f"psT{gi}", bufs=2, space="PSUM") as psT,
                    tc.tile_pool(name=f"psV{gi}", bufs=2, space="PSUM") as psV,
                ):
                    for t in range(NT):
                        x2r = []
                        for s in range(NS):
                            sch2 = spool.tile([P, NT, P], dt.bfloat16, tag="schunk2")
                            nc.sync.dma_start(out=sch2, in_=s2_d.ap()[s, t])
                            ps2 = psB.tile([P, F, BL], dt.float32, tag="ps_x2")
                            spmm_to_psum(ps2, sch2, x1_t[s], inp_part=is_ru)
                            x2s = tpool.tile([P, F, BL], dt.bfloat16, tag=f"x2_{s}")
                            if is_ru:
                                nc.vector.tensor_sub(x2s, ps2, x0_t[t])
                                nc.vector.tensor_copy(
                                    out=x2i_t[s][t], in_=x2s[:, 64:F, :]
                                )
                            else:
                                nc.vector.tensor_sub(
                                    x2s[:, 0:64, :], ps2[:, 0:64, :], x0_t[t][:, 0:64, :]
                                )
                                nc.vector.tensor_copy(
                                    out=x2s[:, 64:F, :], in_=x2i_t[s][t]
                                )
                            x2r.append(x2s)

                        # order must match W row order: x0, x1_s0, x2_s0, x1_s1, x2_s1
                        msrc = [x0_t[t], x1_t[0][t], x2r[0], x1_t[1][t], x2r[1]]
                        xT = []
                        for m in range(5):
                            pT = psT.tile([F, BL, P], dt.bfloat16, tag="psT")
                            for b in range(BL):
                                nc.tensor.transpose(pT[:, b, :], msrc[m][:, :, b], ident)
                            xm = tpool.tile([F, BL, P], dt.bfloat16, tag="xT", bufs=6)
                            eng = nc.scalar.copy if (m % 2 == 0) else nc.vector.tensor_copy
                            eng(out=xm, in_=pT)
                            xT.append(xm)

                        odim = 2 * NU if is_ru else NU
                        wmat = wru if is_ru else wc
                        if is_ru:
                            u_nm = tpool.tile([P, BL, NU], dt.float32, tag="u_nm")
                            for h in range(2):
                                psv = psV.tile([odim, 4, P], dt.float32, tag="psv")
                                for m in range(5):
                                    nc.tensor.matmul(
                                        psv,
                                        wmat[:, m, :],
                                        xT[m][:, 4 * h : 4 * h + 4, :],
                                        start=(m == 0),
                                        stop=(m == 4),
                                    )
                                sig = tpool.tile([odim, 4, P], dt.bfloat16, tag="sig")
                                nc.scalar.activation(sig, psv, AF.Sigmoid, bias=biasru)
                                for j in range(4):
                                    b = 4 * h + j
                                    # r -> x0 *= r (in place: becomes gconv2 input)
                                    pr = psT.tile([P, NU, 1], dt.bfloat16, tag="psT")
                                    nc.tensor.transpose(
                                        pr[:, :, 0], sig[0:NU, j, :], ident[0:NU, 0:NU]
                                    )
                                    nc.vector.tensor_mul(
                                        x0_t[t][:, 0:NU, b], x0_t[t][:, 0:NU, b], pr[:, :, 0]
                                    )
                                    # u -> spill (fp32) for the final combine
                                    pu = psT.tile([P, NU, 1], dt.bfloat16, tag="psT")
                                    nc.tensor.transpose(
                                        pu[:, :, 0], sig[NU : 2 * NU, j, :], ident[NU:P, NU:P]
                                    )
                                    nc.scalar.copy(out=u_nm[:, b, :], in_=pu[:, :, 0])
                            nc.sync.dma_start(out=usp[t], in_=u_nm)
                        else:
                            c_nm = tpool.tile([P, BL, NU], dt.float32, tag="c_nm")
                            for h in range(2):
                                psv = psV.tile([odim, 4, P], dt.float32, tag="psv")
                                for m in range(5):
                                    nc.tensor.matmul(
                                        psv,
                                        wmat[:, m, :],
                                        xT[m][:, 4 * h : 4 * h + 4, :],
                                        start=(m == 0),
                                        stop=(m == 4),
                                    )
                                sigc = tpool.tile([odim, 4, P], dt.bfloat16, tag="sig")
                                nc.scalar.activation(sigc, psv, AF.Tanh, bias=biasc)
                                for j in range(4):
                                    b = 4 * h + j
                                    pc = psT.tile([P, NU, 1], dt.bfloat16, tag="psT")
                                    nc.tensor.transpose(
                                        pc[:, :, 0], sigc[:, j, :], ident[0:NU, 0:NU]
                                    )
                                    nc.scalar.copy(out=c_nm[:, b, :], in_=pc[:, :, 0])
                            # final: out = c + u * (hx - c)
                            hxt = tpool.tile([P, BL, NU], dt.float32, tag="hxt")
                            nc.sync.dma_start(
                                out=hxt, in_=hx_d.ap()[t * P : (t + 1) * P]
                            )
                            ut = tpool.tile([P, BL, NU], dt.float32, tag="ut")
                            nc.sync.dma_start(out=ut, in_=usp[t])
                            tmp = tpool.tile([P, BL, NU], dt.float32, tag="tmp")
                            nc.vector.tensor_sub(tmp, hxt, c_nm)
                            nc.vector.tensor_mul(tmp, tmp, ut)
                            osb = tpool.tile([P, BL, NU], dt.float32, tag="osb")
                            nc.vector.tensor_add(osb, tmp, c_nm)
                            dst = bass.AP(
                                out_d,
                                t * P * NU,
                                [[NU, P], [N * NU, BL], [1, NU]],
                            )
                            nc.sync.dma_start(out=dst, in_=osb)

            phases = getattr(sys.modules[__name__], "_PHASES", [0, 1, 2, 3])
            if 0 in phases:
                pass_a(0)
            if 1 in phases:
                pass_b(0)
            if 2 in phases:
                pass_a(1)
            if 3 in phases:
                pass_b(1)

    nc.finalize()
    return nc


def _pack_core(inputs_f32, hx_f32, b0):
    """Host-side per-core input packing (layout only, no model math).

    Feature order is [state(0:64), inputs(64:66)] so the 512/16 matmul column
    split coincides with the state/input boundary (the input features are
    identical in both gconvs and their diffusion is reused).
    """
    inp = inputs_f32[b0 : b0 + BL].reshape(BL, N, 2).transpose(1, 2, 0)  # [N,2,BL]
    hxs = hx_f32[b0 : b0 + BL].reshape(BL, N, NU)
    hx_f = hxs.transpose(1, 2, 0)  # [N,64,BL]
    x0 = np.concatenate([hx_f, inp], axis=1).astype(BF16)  # [N,66,BL]
    hx_nm = np.ascontiguousarray(hxs.transpose(1, 0, 2), dtype=np.float32)  # [N,BL,64]
    return x0, hx_nm


# feature permutation matching _pack_core: reference order [inputs(2), state(64)]
# -> kernel order [state(64), inputs(2)]
_FPERM = np.concatenate([np.arange(2, F), np.arange(2)])


def make_in_maps(inputs, hx, sup_rows, sup_cols, sup_vals, W_ru, b_ru, W_c, b_c):
    inputs = np.asarray(inputs, dtype=np.float32)
    hx = np.asarray(hx, dtype=np.float32)
    sup_rows = np.asarray(sup_rows)
    sup_cols = np.asarray(sup_cols)
    sup_vals = np.asarray(sup_vals, dtype=np.float32)

    # densify supports (duplicate (r,c) pairs accumulate, matching segment_sum)
    s_dense = np.zeros((NS, N, N), dtype=np.float32)
    for s in range(NS):
        np.add.at(s_dense[s], (sup_rows[s], sup_cols[s]), sup_vals[s])

    def blockify(mat):
        # stationary layout: [dst, c_part, src, n] with
        # block[d, i, k, j] = mat[128d + j, 128k + i]  (= mat^T[c, n] blocks)
        t = mat.T.reshape(NT, P, NT, P)  # [k, i, d, j]
        return np.ascontiguousarray(
            t.transpose(2, 1, 0, 3).reshape(NT, P, NT * P)
        ).astype(BF16)

    s1 = np.stack([blockify(s_dense[s]) for s in range(NS)])
    s2 = np.stack([blockify(2.0 * s_dense[s]) for s in range(NS)])

    # W rows are interleaved (f, m) -> reorder to [f][m][o], with f permuted to
    # the kernel's [state, inputs] feature order
    wru = np.ascontiguousarray(
        np.asarray(W_ru, dtype=np.float32).reshape(F, 5, 2 * NU)[_FPERM]
    ).astype(BF16)
    wc = np.ascontiguousarray(
        np.asarray(W_c, dtype=np.float32).reshape(F, 5, NU)[_FPERM]
    ).astype(BF16)
    bru = np.asarray(b_ru, dtype=np.float32).reshape(2 * NU, 1)
    bc = np.asarray(b_c, dtype=np.float32).reshape(NU, 1)

    in_maps = []
    for c in range(NC):
        x0, hx_nm = _pack_core(inputs, hx, c * BL)
        in_maps.append(
            {
                "x0in": x0,
                "hxn": hx_nm,
                "s1": s1,
                "s2": s2,
                "wru": wru,
                "wc": wc,
                "bru": bru,
                "bc": bc,
            }
        )
    return in_maps


def kernel(inputs, hx, sup_rows, sup_cols, sup_vals, W_ru, b_ru, W_c, b_c):
    global LAST_RESULTS, _NC_CACHE

    in_maps = make_in_maps(
        inputs, hx, sup_rows, sup_cols, sup_vals, W_ru, b_ru, W_c, b_c
    )

    if _NC_CACHE is None:
        _NC_CACHE = _build_nc()
    nc = _NC_CACHE

    res = run_bass_kernel_spmd(nc, in_maps, core_ids=list(range(NC)), trace=False)
    LAST_RESULTS = res

    out = np.concatenate(
        [np.asarray(res.results[c]["out"], dtype=np.float32) for c in range(NC)],
        axis=0,
    )
    return out
